# revision 4
# baseline (speedup 1.0000x reference)
"""HTSK fuzzy-system kernel for Trainium2 (Bass/Tile), 8-core data-parallel.

Math (per batch row b):
  S     = H/sigma^2 + EPS                          (D,R)
  m     = mean_d(-(X_bd - C_dr)^2 * S_dr)          (B,R)
        = X^2 @ A + X @ Bm + 1*K2                  (matmul expansion)
  e_n   = softmax_r(m)                             (normalized firing)
  G     = X @ Wt',  Wt'[d, o*R+r] = W[r*D+d, o]    (o-major columns)
  out   = sum_r e_n[b,r]*G[b,o,r]  +  e_n @ (W2 + 1 b^T)

Schedule: per 128-row tile, PE streams G into PSUM in 1024-col groups,
ACT evicts psum->bf16, DVE (2x mode, o-major keeps unit stride under the
e-broadcast) multiplies by e_n and runs most of the halving tree, Pool
(gpsimd) takes the o>=48 multiply and the o>=52 tree. Phase-1 (softmax)
of tile t+2 is software-pipelined into tile t's G phase.

Sharding: batch B=4096 split 512 rows per core; weights replicated.
"""
import sys
import types
from contextlib import ExitStack

import numpy as np

sys.path.insert(0, "/opt/trn_rl_repo")

# NTFF profile-hook registry: trn_boot §6 sets it at jax init, concourse
# bass_utils reads it when trace=True. The container's antenv package lacks
# this submodule, so provide it before anything imports jax/concourse.
if "antenv.axon_hooks" not in sys.modules:
    _ah = types.ModuleType("antenv.axon_hooks")
    _ah._hook = None

    def _set_hook(hook):
        _ah._hook = hook

    def _get_hook():
        return _ah._hook

    _ah.set_axon_ntff_profile_hook = _set_hook
    _ah.get_axon_ntff_profile_hook = _get_hook
    sys.modules["antenv.axon_hooks"] = _ah

import ml_dtypes  # noqa: E402
import concourse.bass as bass  # noqa: E402
import concourse.bacc as bacc  # noqa: E402
import concourse.tile as tile  # noqa: E402
from concourse import mybir  # noqa: E402
from concourse import bass_utils  # noqa: E402
from concourse.masks import make_identity  # noqa: E402

H = 0.5
EPS = 1e-8
B, D, R, O = 4096, 256, 128, 64
NCORES = 8
BL = B // NCORES          # 512 batch rows per core
NT = BL // 128            # 4 partition tiles per core
RO = R * O                # 8192
NG = 8                    # 1024-col (8-o) G groups per tile
O_MUL = 48                # DVE multiplies o < O_MUL, Pool the rest
O_TREE = 52               # DVE tree o < O_TREE, Pool the rest
F32 = mybir.dt.float32
BF16 = mybir.dt.bfloat16

_CACHE = {}


def _build():
    nc = bacc.Bacc("TRN2", target_bir_lowering=False, debug=False)
    X = nc.dram_tensor("X", [BL, D], BF16, kind="ExternalInput")
    A = nc.dram_tensor("A", [D, R], BF16, kind="ExternalInput")
    Bm = nc.dram_tensor("Bm", [D, R], BF16, kind="ExternalInput")
    K2 = nc.dram_tensor("K2", [1, R], BF16, kind="ExternalInput")
    W2p = nc.dram_tensor("W2p", [R, O], BF16, kind="ExternalInput")
    Wt = nc.dram_tensor("Wt", [D, RO], BF16, kind="ExternalInput")
    out = nc.dram_tensor("out", [BL, O], F32, kind="ExternalOutput")

    with tile.TileContext(nc) as tc, ExitStack() as ctx:
        consts = ctx.enter_context(tc.tile_pool(name="consts", bufs=1))
        xpool = ctx.enter_context(tc.tile_pool(name="xp", bufs=4))
        ph1p = ctx.enter_context(tc.tile_pool(name="ph1", bufs=2))
        epool = ctx.enter_context(tc.tile_pool(name="ep", bufs=3))
        gsbp = ctx.enter_context(tc.tile_pool(name="gsb", bufs=2))
        gmwp = ctx.enter_context(tc.tile_pool(name="gmw", bufs=2))
        trp = ctx.enter_context(tc.tile_pool(name="tr", bufs=2))
        osbp = ctx.enter_context(tc.tile_pool(name="osb", bufs=2))
        ps_s = ctx.enter_context(tc.tile_pool(name="ps_s", bufs=1, space="PSUM"))
        ps_g = ctx.enter_context(tc.tile_pool(name="ps_g", bufs=3, space="PSUM"))

        # ---- constants; small ones + X tiles on the scalar HWDGE queue,
        # Wt streams in G-group order on the sync HWDGE queue ----
        ones_sb = consts.tile([1, 128], BF16, tag="ones")
        nc.vector.memset(ones_sb, 1.0)
        a_sb = consts.tile([128, 2, R], BF16, tag="a")
        bm_sb = consts.tile([128, 2, R], BF16, tag="bm")
        for c in range(2):
            nc.scalar.dma_start(out=a_sb[:, c, :], in_=A[c * 128:(c + 1) * 128, :])
            nc.scalar.dma_start(out=bm_sb[:, c, :], in_=Bm[c * 128:(c + 1) * 128, :])
        k2_sb = consts.tile([1, R], BF16, tag="k2")
        nc.scalar.dma_start(out=k2_sb[:, :], in_=K2[:, :])
        w2p_sb = consts.tile([R, O], BF16, tag="w2p")
        nc.scalar.dma_start(out=w2p_sb[:, :], in_=W2p[:, :])
        xts = []
        for t in range(NT):
            xt = xpool.tile([128, D], BF16, tag="xt", name=f"xt{t}")
            nc.scalar.dma_start(out=xt[:, :], in_=X[t * 128:(t + 1) * 128, :])
            xts.append(xt)
        wt_sb = [[None] * NG for _ in range(2)]
        for g in range(NG):
            for c in range(2):
                w_ = consts.tile([128, 1024], BF16, tag=f"wt{c}{g}")
                nc.sync.dma_start(
                    out=w_[:, :],
                    in_=Wt[c * 128:(c + 1) * 128, g * 1024:(g + 1) * 1024],
                )
                wt_sb[c][g] = w_

        def ph1(t):
            """Transpose + membership logits + normalized softmax weights."""
            xTb = ph1p.tile([128, 2, 128], BF16, tag="xTb", name=f"xTb{t}")
            for c in range(2):
                nc.scalar.dma_start(
                    out=xTb[:, c, :], in_=xts[t][:, c * 128:(c + 1) * 128],
                    transpose=True,
                )
            x2Tb = ph1p.tile([128, 2, 128], BF16, tag="x2Tb", name=f"x2Tb{t}")
            nc.scalar.activation(x2Tb, xTb, mybir.ActivationFunctionType.Square)
            m_ps = ps_s.tile([128, R], F32, tag="m", name=f"m{t}")
            nc.tensor.matmul(m_ps, lhsT=x2Tb[:, 0, :], rhs=a_sb[:, 0, :],
                             start=True, stop=False)
            nc.tensor.matmul(m_ps, lhsT=x2Tb[:, 1, :], rhs=a_sb[:, 1, :],
                             start=False, stop=False)
            nc.tensor.matmul(m_ps, lhsT=xTb[:, 0, :], rhs=bm_sb[:, 0, :],
                             start=False, stop=False)
            nc.tensor.matmul(m_ps, lhsT=xTb[:, 1, :], rhs=bm_sb[:, 1, :],
                             start=False, stop=False)
            nc.tensor.matmul(m_ps, lhsT=ones_sb, rhs=k2_sb,
                             start=False, stop=True)
            nmx = ph1p.tile([128, 1], F32, tag="nmx", name=f"nmx{t}")
            nc.vector.reduce_max(nmx, m_ps, axis=mybir.AxisListType.X, negate=True)
            e_raw = ph1p.tile([128, R], BF16, tag="eraw", name=f"eraw{t}")
            s_ = ph1p.tile([128, 1], F32, tag="s", name=f"s{t}")
            nc.scalar.activation(e_raw, m_ps, mybir.ActivationFunctionType.Exp,
                                 bias=nmx, scale=1.0, accum_out=s_)
            rs = ph1p.tile([128, 1], F32, tag="rs", name=f"rs{t}")
            nc.vector.reciprocal(rs, s_)
            e_n = epool.tile([128, R], BF16, tag="en", name=f"en{t}")
            nc.vector.tensor_scalar_mul(e_n, e_raw, rs)
            eTs = ph1p.tile([128, 128], BF16, tag="eTs", name=f"eTs{t}")
            nc.scalar.dma_start(out=eTs, in_=e_n, transpose=True)
            o2p = ps_s.tile([128, O], F32, tag="o2", name=f"o2{t}")
            nc.tensor.matmul(o2p, lhsT=eTs, rhs=w2p_sb, start=True, stop=True)
            o2s = epool.tile([128, O], F32, tag="o2s", name=f"o2s{t}")
            nc.scalar.copy(o2s, o2p)
            return xTb, e_n, o2s

        def ph2(t, xTb, e_n, o2s):
            """G matmul stream + e-weighting + r-reduction tree + combine."""
            gsb = gsbp.tile([128, RO], BF16, tag="gsb", name=f"gsb{t}")
            gmw = gmwp.tile([128, O, R], BF16, tag="gmw", name=f"gmw{t}")
            ebc16 = e_n.rearrange("p r -> p () r").broadcast_to((128, 16, R))
            for g in range(NG):
                gt = ps_g.tile([128, 1024], F32, tag="g", name=f"g{t}_{g}")
                for h in range(2):
                    for c in range(2):
                        nc.tensor.matmul(
                            gt[:, h * 512:(h + 1) * 512],
                            lhsT=xTb[:, c, :],
                            rhs=wt_sb[c][g][:, h * 512:(h + 1) * 512],
                            start=(c == 0), stop=(c == 1),
                        )
                nc.scalar.copy(gsb[:, g * 1024:(g + 1) * 1024], gt)
                if g % 2 == 1 and g < 6:
                    # 16-o weighted multiply on DVE (bf16 2x mode)
                    k = g // 2
                    gv = gsb[:, k * 2048:(k + 1) * 2048].rearrange(
                        "p (o r) -> p o r", r=R
                    )
                    nc.vector.tensor_mul(gmw[:, 16 * k:16 * (k + 1), :], gv, ebc16)
                if g == 7:
                    # o 48:64 weighted multiply on Pool
                    gv = gsb[:, 6144:8192].rearrange("p (o r) -> p o r", r=R)
                    nc.gpsimd.tensor_mul(gmw[:, 48:64, :], gv, ebc16)
            return gsb, gmw

        def trees(t, gmw, o2s):
            """Halving tree over r, split o<O_TREE on DVE / rest on Pool."""
            osb = osbp.tile([128, O], F32, tag="osb", name=f"osb{t}")
            for (eng, lo, hi, nm) in (
                (nc.vector, 0, O_TREE, "D"),
                (nc.gpsimd, O_TREE, O, "P"),
            ):
                n_o = hi - lo
                cur = gmw[:, lo:hi, :]
                w = R
                while w > 2:
                    w2 = w // 2
                    nxt = trp.tile([128, n_o, w2], BF16, tag=f"tr{nm}{w2}",
                                   name=f"tr{nm}{w2}_{t}")
                    eng.tensor_add(nxt, cur[:, :, 0:w2], cur[:, :, w2:w])
                    cur = nxt
                    w = w2
                red = trp.tile([128, n_o], F32, tag=f"red{nm}",
                               name=f"red{nm}_{t}")
                eng.tensor_add(red, cur[:, :, 0:1], cur[:, :, 1:2])
                eng.tensor_add(osb[:, lo:hi], red, o2s[:, lo:hi])
            nc.sync.dma_start(out=out[t * 128:(t + 1) * 128, :], in_=osb)

        ctx_t = [ph1(0), ph1(1)]
        for t in range(NT):
            xTb, e_n, o2s = ctx_t[t]
            gsb, gmw = ph2(t, xTb, e_n, o2s)
            if t + 2 < NT:
                ctx_t.append(ph1(t + 2))
            trees(t, gmw, o2s)

    nc.finalize()
    return nc


def _get_nc():
    if "nc" not in _CACHE:
        _CACHE["nc"] = _build()
    return _CACHE["nc"]


def _host_prep(centers, sigmas, W, b):
    c64 = centers.astype(np.float64)
    S = (H / sigmas.astype(np.float64) ** 2) + EPS          # (D,R)
    A = (-S / D).astype(ml_dtypes.bfloat16)                  # X^2 coeff
    Bm = (2.0 * S * c64 / D).astype(ml_dtypes.bfloat16)      # X coeff
    K2 = (-(S * c64 * c64).sum(axis=0, keepdims=True) / D).astype(
        ml_dtypes.bfloat16
    )
    W1 = W[: D * R].reshape(R, D, O)
    # o-major columns: Wt[d, o*R + r] = W1[r, d, o]
    Wt = np.ascontiguousarray(W1.transpose(1, 2, 0).reshape(D, RO)).astype(
        ml_dtypes.bfloat16
    )
    W2p = (W[D * R:].astype(np.float64) + b[None, :].astype(np.float64)).astype(
        ml_dtypes.bfloat16
    )
    return A, Bm, K2, W2p, Wt


def kernel(X, centers, sigmas, W, b):
    X = np.asarray(X, dtype=np.float32)
    centers = np.asarray(centers, dtype=np.float32)
    sigmas = np.asarray(sigmas, dtype=np.float32)
    W = np.asarray(W, dtype=np.float32)
    b = np.asarray(b, dtype=np.float32)

    A, Bm, K2, W2p, Wt = _host_prep(centers, sigmas, W, b)
    Xb = X.astype(ml_dtypes.bfloat16)
    nc = _get_nc()
    in_maps = [
        {
            "X": np.ascontiguousarray(Xb[k * BL:(k + 1) * BL]),
            "A": A, "Bm": Bm, "K2": K2, "W2p": W2p, "Wt": Wt,
        }
        for k in range(NCORES)
    ]
    res = bass_utils.run_bass_kernel_spmd(nc, in_maps, core_ids=list(range(NCORES)))
    return np.concatenate([res.results[k]["out"] for k in range(NCORES)], axis=0)


# revision 8
# speedup vs baseline: 1.2220x; 1.2220x over previous
"""HTSK fuzzy-system kernel for Trainium2 (Bass/Tile), 8-core data-parallel.

Math (per batch row b):
  S     = H/sigma^2 + EPS                          (D,R)
  m     = mean_d(-(X_bd - C_dr)^2 * S_dr)          (B,R)
        = X^2 @ A + X @ Bm + 1*K2                  (matmul expansion)
  e_n   = softmax_r(m)                             (normalized firing)
  G     = X @ Wt',  Wt'[d, o*R+r] = W[r*D+d, o]    (o-major columns)
  out   = sum_r e_n[b,r]*G[b,o,r]  +  e_n @ (W2 + 1 b^T)

Schedule: per 128-row tile, PE streams G into PSUM in 1024-col groups,
ACT evicts psum->bf16, DVE (2x mode; o-major keeps unit stride under the
e-broadcast) multiplies o<40 by e_n, Pool multiplies o>=40 (large
contiguous op only - Pool is slow on small/strided work), DVE runs a
3-level halving tree + segmented tensor_reduce. Phase-1 (softmax) of
tile t+2 is software-pipelined behind tile t's G phase. All DMAs issue
from the sync engine; constants ride in one packed tensor.

Sharding: batch B=4096 split 512 rows per core; weights replicated.
"""
import sys
import types
from contextlib import ExitStack

import numpy as np

sys.path.insert(0, "/opt/trn_rl_repo")

# NTFF profile-hook registry: trn_boot §6 sets it at jax init, concourse
# bass_utils reads it when trace=True. The container's antenv package lacks
# this submodule, so provide it before anything imports jax/concourse.
if "antenv.axon_hooks" not in sys.modules:
    _ah = types.ModuleType("antenv.axon_hooks")
    _ah._hook = None

    def _set_hook(hook):
        _ah._hook = hook

    def _get_hook():
        return _ah._hook

    _ah.set_axon_ntff_profile_hook = _set_hook
    _ah.get_axon_ntff_profile_hook = _get_hook
    sys.modules["antenv.axon_hooks"] = _ah

import ml_dtypes  # noqa: E402
import concourse.bass as bass  # noqa: E402
import concourse.bacc as bacc  # noqa: E402
import concourse.tile as tile  # noqa: E402
from concourse import mybir  # noqa: E402
from concourse import bass_utils  # noqa: E402
from concourse.masks import make_identity  # noqa: E402

H = 0.5
EPS = 1e-8
B, D, R, O = 4096, 256, 128, 64
NCORES = 8
BL = B // NCORES          # 512 batch rows per core
NT = BL // 128            # 4 partition tiles per core
RO = R * O                # 8192
NG = 8                    # 1024-col (8-o) G groups per tile
O_MUL = 40                # DVE multiplies o < O_MUL, Pool the rest
F32 = mybir.dt.float32
BF16 = mybir.dt.bfloat16

_CACHE = {}


def _build():
    nc = bacc.Bacc("TRN2", target_bir_lowering=False, debug=False)
    X = nc.dram_tensor("X", [BL, D], BF16, kind="ExternalInput")
    # packed consts: [A_c0 | A_c1 | Bm_c0 | Bm_c1 | W2p] along columns
    PK = nc.dram_tensor("PK", [128, 4 * R + O], BF16, kind="ExternalInput")
    K2 = nc.dram_tensor("K2", [1, R], BF16, kind="ExternalInput")
    Wt = nc.dram_tensor("Wt", [D, RO], BF16, kind="ExternalInput")
    out = nc.dram_tensor("out", [BL, O], F32, kind="ExternalOutput")

    with tile.TileContext(nc) as tc, ExitStack() as ctx:
        consts = ctx.enter_context(tc.tile_pool(name="consts", bufs=1))
        ph1p = ctx.enter_context(tc.tile_pool(name="ph1", bufs=2))
        epool = ctx.enter_context(tc.tile_pool(name="ep", bufs=3))
        gsbp = ctx.enter_context(tc.tile_pool(name="gsb", bufs=2))
        gmwp = ctx.enter_context(tc.tile_pool(name="gmw", bufs=2))
        trp = ctx.enter_context(tc.tile_pool(name="tr", bufs=2))
        osbp = ctx.enter_context(tc.tile_pool(name="osb", bufs=2))
        ps_m = ctx.enter_context(tc.tile_pool(name="ps_m", bufs=1, space="PSUM"))
        ps_a = ctx.enter_context(tc.tile_pool(name="ps_a", bufs=1, space="PSUM"))
        ps_g = ctx.enter_context(tc.tile_pool(name="ps_g", bufs=3, space="PSUM"))

        # ---- constants + X, all on the sync HWDGE queue; Wt streams in
        # G-group order behind them ----
        identB = consts.tile([128, 128], BF16, tag="idb")
        make_identity(nc, identB)
        ones_sb = consts.tile([1, 128], BF16, tag="ones")
        nc.vector.memset(ones_sb, 1.0)
        pk_sb = consts.tile([128, 4 * R + O], BF16, tag="pk")
        nc.sync.dma_start(out=pk_sb, in_=PK[:, :])
        k2_sb = consts.tile([1, R], BF16, tag="k2")
        nc.sync.dma_start(out=k2_sb[:, :], in_=K2[:, :])
        xall = consts.tile([128, NT, D], BF16, tag="xall")
        nc.sync.dma_start(
            out=xall, in_=X[:, :].rearrange("(t p) d -> p t d", t=NT)
        )
        a_rhs = [pk_sb[:, 0:R], pk_sb[:, R:2 * R]]
        bm_rhs = [pk_sb[:, 2 * R:3 * R], pk_sb[:, 3 * R:4 * R]]
        w2p_sb = pk_sb[:, 4 * R:4 * R + O]
        wt_sb = [[None] * 4 for _ in range(2)]
        for q in range(4):
            for c in range(2):
                w_ = consts.tile([128, 2048], BF16, tag=f"wt{c}{q}")
                nc.sync.dma_start(
                    out=w_[:, :],
                    in_=Wt[c * 128:(c + 1) * 128, q * 2048:(q + 1) * 2048],
                )
                wt_sb[c][q] = w_

        def ph1(t):
            """Transpose + membership logits + normalized softmax weights."""
            xtT = ps_a.tile([128, 2, 128], BF16, tag="aux", name=f"xtT{t}")
            for c in range(2):
                nc.tensor.transpose(
                    xtT[:, c, :], xall[:, t, c * 128:(c + 1) * 128], identB
                )
            xTb = ph1p.tile([128, 2, 128], BF16, tag="xTb", name=f"xTb{t}")
            x2Tb = ph1p.tile([128, 2, 128], BF16, tag="x2Tb", name=f"x2Tb{t}")
            nc.vector.tensor_copy(xTb, xtT)
            nc.scalar.activation(x2Tb, xtT, mybir.ActivationFunctionType.Square)
            m_ps = ps_m.tile([128, R], F32, tag="m", name=f"m{t}")
            nc.tensor.matmul(m_ps, lhsT=x2Tb[:, 0, :], rhs=a_rhs[0],
                             start=True, stop=False)
            nc.tensor.matmul(m_ps, lhsT=x2Tb[:, 1, :], rhs=a_rhs[1],
                             start=False, stop=False)
            nc.tensor.matmul(m_ps, lhsT=xTb[:, 0, :], rhs=bm_rhs[0],
                             start=False, stop=False)
            nc.tensor.matmul(m_ps, lhsT=xTb[:, 1, :], rhs=bm_rhs[1],
                             start=False, stop=False)
            nc.tensor.matmul(m_ps, lhsT=ones_sb, rhs=k2_sb,
                             start=False, stop=True)
            nmx = ph1p.tile([128, 1], F32, tag="nmx", name=f"nmx{t}")
            nc.vector.reduce_max(nmx, m_ps, axis=mybir.AxisListType.X, negate=True)
            e_raw = ph1p.tile([128, R], BF16, tag="eraw", name=f"eraw{t}")
            s_ = ph1p.tile([128, 1], F32, tag="s", name=f"s{t}")
            nc.scalar.activation(e_raw, m_ps, mybir.ActivationFunctionType.Exp,
                                 bias=nmx, scale=1.0, accum_out=s_)
            rs = ph1p.tile([128, 1], F32, tag="rs", name=f"rs{t}")
            nc.vector.reciprocal(rs, s_)
            e_n = epool.tile([128, R], BF16, tag="en", name=f"en{t}")
            nc.vector.tensor_scalar_mul(e_n, e_raw, rs)
            eT = ps_a.tile([128, 128], BF16, tag="aux", name=f"eT{t}")
            nc.tensor.transpose(eT, e_n, identB)
            eTs = ph1p.tile([128, 128], BF16, tag="eTs", name=f"eTs{t}")
            nc.vector.tensor_copy(eTs, eT)
            o2p = ps_a.tile([128, O], F32, tag="aux", name=f"o2{t}")
            nc.tensor.matmul(o2p, lhsT=eTs, rhs=w2p_sb, start=True, stop=True)
            o2s = epool.tile([128, O], F32, tag="o2s", name=f"o2s{t}")
            nc.scalar.copy(o2s, o2p)
            return xTb, e_n, o2s

        def ph2(t, xTb, e_n, o2s):
            """G matmul stream + e-weighting + partial tree."""
            gsb = gsbp.tile([128, RO], BF16, tag="gsb", name=f"gsb{t}")
            gmw = gmwp.tile([128, O, R], BF16, tag="gmw", name=f"gmw{t}")
            ebc16 = e_n.rearrange("p r -> p () r").broadcast_to((128, 16, R))
            ebc8 = e_n.rearrange("p r -> p () r").broadcast_to((128, 8, R))
            ebc24 = e_n.rearrange("p r -> p () r").broadcast_to((128, 24, R))
            tr64 = trp.tile([128, O, 64], BF16, tag="t64", name=f"t64_{t}")
            for g in range(NG):
                gt = ps_g.tile([128, 1024], F32, tag="g", name=f"g_{t}_{g}")
                for h in range(2):
                    for c in range(2):
                        off = (g % 2) * 1024 + h * 512
                        nc.tensor.matmul(
                            gt[:, h * 512:(h + 1) * 512],
                            lhsT=xTb[:, c, :],
                            rhs=wt_sb[c][g // 2][:, off:off + 512],
                            start=(c == 0), stop=(c == 1),
                        )
                nc.scalar.copy(gsb[:, g * 1024:(g + 1) * 1024], gt)
                if g in (1, 3):
                    # 16-o weighted multiply on DVE (bf16 2x mode)
                    k = g // 2
                    gv = gsb[:, k * 2048:(k + 1) * 2048].rearrange(
                        "p (o r) -> p o r", r=R
                    )
                    nc.vector.tensor_mul(gmw[:, 16 * k:16 * (k + 1), :], gv, ebc16)
                if g == 3:
                    # first tree level for o 0:32 while later groups stream
                    nc.vector.tensor_add(
                        tr64[:, 0:32, :], gmw[:, 0:32, 0:64], gmw[:, 0:32, 64:R]
                    )
                if g == 4:
                    gv = gsb[:, 4096:5120].rearrange("p (o r) -> p o r", r=R)
                    nc.vector.tensor_mul(gmw[:, 32:40, :], gv, ebc8)
                if g == 7:
                    # o 40:64 weighted multiply on Pool (large contiguous op)
                    gv = gsb[:, O_MUL * R:].rearrange("p (o r) -> p o r", r=R)
                    nc.gpsimd.tensor_mul(gmw[:, O_MUL:, :], gv, ebc24)
            return gsb, gmw, tr64

        def trees(t, gmw, tr64, o2s):
            """Rest of the r-reduction on DVE + combine + store."""
            osb = osbp.tile([128, O], F32, tag="osb", name=f"osb{t}")
            nc.vector.tensor_add(
                tr64[:, 32:O, :], gmw[:, 32:O, 0:64], gmw[:, 32:O, 64:R]
            )
            tr32 = trp.tile([128, O, 32], BF16, tag="t32", name=f"t32_{t}")
            nc.vector.tensor_add(tr32, tr64[:, :, 0:32], tr64[:, :, 32:64])
            tr16 = trp.tile([128, O, 16], BF16, tag="t16", name=f"t16_{t}")
            nc.vector.tensor_add(tr16, tr32[:, :, 0:16], tr32[:, :, 16:32])
            red = trp.tile([128, O], F32, tag="red", name=f"red{t}")
            nc.vector.reduce_sum(red, tr16, axis=mybir.AxisListType.X)
            nc.vector.tensor_add(osb, red, o2s)
            nc.sync.dma_start(out=out[t * 128:(t + 1) * 128, :], in_=osb)

        ctx_t = [ph1(0), ph1(1)]
        for t in range(NT):
            xTb, e_n, o2s = ctx_t[t]
            gsb, gmw, tr64 = ph2(t, xTb, e_n, o2s)
            trees(t, gmw, tr64, o2s)
            if t + 2 < NT:
                ctx_t.append(ph1(t + 2))

    nc.finalize()
    return nc


def _get_nc():
    if "nc" not in _CACHE:
        _CACHE["nc"] = _build()
    return _CACHE["nc"]


def _host_prep(centers, sigmas, W, b):
    c64 = centers.astype(np.float64)
    S = (H / sigmas.astype(np.float64) ** 2) + EPS          # (D,R)
    A = (-S / D).astype(ml_dtypes.bfloat16)                  # X^2 coeff
    Bm = (2.0 * S * c64 / D).astype(ml_dtypes.bfloat16)      # X coeff
    K2 = (-(S * c64 * c64).sum(axis=0, keepdims=True) / D).astype(
        ml_dtypes.bfloat16
    )
    W1 = W[: D * R].reshape(R, D, O)
    # o-major columns: Wt[d, o*R + r] = W1[r, d, o]
    Wt = np.ascontiguousarray(W1.transpose(1, 2, 0).reshape(D, RO)).astype(
        ml_dtypes.bfloat16
    )
    W2p = (W[D * R:].astype(np.float64) + b[None, :].astype(np.float64)).astype(
        ml_dtypes.bfloat16
    )
    # packed consts: [A_c0 | A_c1 | Bm_c0 | Bm_c1 | W2p]
    PK = np.concatenate(
        [np.asarray(A[0:128]), np.asarray(A[128:256]),
         np.asarray(Bm[0:128]), np.asarray(Bm[128:256]), np.asarray(W2p)],
        axis=1,
    ).astype(ml_dtypes.bfloat16)
    return np.ascontiguousarray(PK), K2, Wt


def kernel(X, centers, sigmas, W, b):
    X = np.asarray(X, dtype=np.float32)
    centers = np.asarray(centers, dtype=np.float32)
    sigmas = np.asarray(sigmas, dtype=np.float32)
    W = np.asarray(W, dtype=np.float32)
    b = np.asarray(b, dtype=np.float32)

    PK, K2, Wt = _host_prep(centers, sigmas, W, b)
    Xb = X.astype(ml_dtypes.bfloat16)
    nc = _get_nc()
    in_maps = [
        {
            "X": np.ascontiguousarray(Xb[k * BL:(k + 1) * BL]),
            "PK": PK, "K2": K2, "Wt": Wt,
        }
        for k in range(NCORES)
    ]
    res = bass_utils.run_bass_kernel_spmd(nc, in_maps, core_ids=list(range(NCORES)))
    return np.concatenate([res.results[k]["out"] for k in range(NCORES)], axis=0)
